# revision 2
# baseline (speedup 1.0000x reference)
"""Trainium2 Bass kernel for the conv-encoder TopK SAE problem.

Math: the reference's conv1d('same', KS=3) followed by sum-over-T pooling
collapses exactly into one matmul:
    pre[b,s] = sum_{j,c} conv_w[s,c,j] * u_j[b,c] + b_enc[s]
with u_1 = sum_t x[b,t,c], u_0 = u_1 - x[b,T-1,c], u_2 = u_1 - x[b,0,c]
(the j-th tap sees the T-window shifted by j-1 under zero padding).

Distribution (8 cores):
  Launch 1 (encode): d_sae sharded; core m holds WencT[:, m*2048:(m+1)*2048]
    (WencT = conv_w reshaped to [3*768, 16384]) and computes
    pre_local = u @ WencT_m as 288 fp32 matmuls with the big operand on the
    LDWEIGHTS (stationary) path. DMA-bound: ~19 MB/core.
  Host: exact top-K of pre (2 MB), builds z, picks the <=B*K needed W_dec rows.
  Launch 2 (decode): batch sharded; core m decodes batches 4m..4m+3 as
    block-diagonal matmuls over only the gathered W_dec rows (6.3 MB/core
    instead of 100 MB/core for a dense decode).
"""

import numpy as np

import concourse.bacc as bacc
import concourse.mybir as mybir
from concourse.tile import TileContext
from concourse.bass_utils import run_bass_kernel_spmd

B, T, D_IN, D_SAE, K, KS = 32, 16, 768, 16384, 32, 3
NCORES = 8
S_SHARD = D_SAE // NCORES          # 2048 features per core
KDIM = KS * D_IN                   # 2304 contraction dim
KT = KDIM // 128                   # 18 k-tiles
ST = S_SHARD // 128                # 16 s-tiles per core
B_SHARD = B // NCORES              # 4 batches per core (decode)
ROWS = B_SHARD * K                 # 128 gathered rows per core (decode)
DFLAT = T * D_IN                   # 12288 decoded features per batch
NCHUNK = DFLAT // 128              # 96 column chunks (decode)
CHUNK_G = 8                        # chunks per DMA group (decode)
NGROUP = NCHUNK // CHUNK_G         # 12 DMA groups (decode)

F32 = mybir.dt.float32

_built = {}


def _build_encode():
    nc = bacc.Bacc("TRN2", target_bir_lowering=False, debug=False, num_devices=NCORES)
    u_d = nc.dram_tensor("u", [KT, 128, B], F32, kind="ExternalInput")
    w_d = nc.dram_tensor("wenc", [KDIM, S_SHARD], F32, kind="ExternalInput")
    o_d = nc.dram_tensor("preT", [ST, 128, B], F32, kind="ExternalOutput")

    with TileContext(nc) as tc:
        with (
            tc.tile_pool(name="wp", bufs=1) as wp,
            tc.tile_pool(name="up", bufs=1) as up,
            tc.tile_pool(name="pp", bufs=1, space="PSUM") as pp,
        ):
            u_sb = up.tile([128, KT, B], F32, tag="u", name="u_sb")
            nc.sync.dma_start(u_sb[:], u_d[:].rearrange("t p b -> p t b"))

            w_sb = []
            for k in range(KT):
                w = wp.tile([128, S_SHARD], F32, tag=f"w{k}", name=f"w{k}")
                nc.sync.dma_start(w[:], w_d[k * 128 : (k + 1) * 128, :])
                w_sb.append(w)

            out_sb = up.tile([128, ST, B], F32, tag="o", name="out_sb")
            # 16 accumulation groups, 8 PSUM banks -> two half passes
            for half in range(2):
                ps = [pp.tile([128, B], F32, tag=f"ps{i}", name=f"ps{half}_{i}") for i in range(8)]
                for k in range(KT):
                    for i in range(8):
                        s = half * 8 + i
                        nc.tensor.matmul(
                            ps[i][:],
                            w_sb[k][:, s * 128 : (s + 1) * 128],
                            u_sb[:, k, :],
                            start=(k == 0),
                            stop=(k == KT - 1),
                        )
                for i in range(8):
                    nc.vector.tensor_copy(out_sb[:, half * 8 + i, :], ps[i][:])

            nc.sync.dma_start(o_d[:].rearrange("s p b -> p s b"), out_sb[:])

    nc.compile()
    return nc


def _build_decode():
    nc = bacc.Bacc("TRN2", target_bir_lowering=False, debug=False, num_devices=NCORES)
    r_d = nc.dram_tensor("rows", [NGROUP, 128, CHUNK_G * 128], F32, kind="ExternalInput")
    z_d = nc.dram_tensor("zdiag", [ROWS, B_SHARD], F32, kind="ExternalInput")
    o_d = nc.dram_tensor("xh", [128, NCHUNK, B_SHARD], F32, kind="ExternalOutput")

    with TileContext(nc) as tc:
        with (
            tc.tile_pool(name="rp", bufs=1) as rp,
            tc.tile_pool(name="sp", bufs=1) as sp,
            tc.tile_pool(name="pp", bufs=8, space="PSUM") as pp,
        ):
            z_sb = sp.tile([ROWS, B_SHARD], F32, tag="z", name="z_sb")
            nc.sync.dma_start(z_sb[:], z_d[:])

            r_sb = []
            for g in range(NGROUP):
                r = rp.tile([128, CHUNK_G * 128], F32, tag=f"r{g}", name=f"r{g}")
                nc.sync.dma_start(r[:], r_d[g])
                r_sb.append(r)

            out_sb = sp.tile([128, NCHUNK, B_SHARD], F32, tag="o", name="out_sb")
            for c in range(NCHUNK):
                g, i = divmod(c, CHUNK_G)
                ps = pp.tile([128, B_SHARD], F32, tag="ps", name=f"ps{c}")
                nc.tensor.matmul(
                    ps[:],
                    r_sb[g][:, i * 128 : (i + 1) * 128],
                    z_sb[:],
                    start=True,
                    stop=True,
                )
                nc.vector.tensor_copy(out_sb[:, c, :], ps[:])

            nc.sync.dma_start(o_d[:], out_sb[:])

    nc.compile()
    return nc


def _get(name):
    if name not in _built:
        _built[name] = _build_encode() if name == "enc" else _build_decode()
    return _built[name]


def kernel(x, conv_w, b_enc, W_dec, b_dec):
    x = np.ascontiguousarray(x, dtype=np.float32)
    conv_w = np.asarray(conv_w, dtype=np.float32)
    b_enc = np.asarray(b_enc, dtype=np.float32)
    W_dec = np.asarray(W_dec, dtype=np.float32)
    b_dec = np.asarray(b_dec, dtype=np.float32)
    core_ids = list(range(NCORES))

    # ---- host prep: collapse conv+pool into u, reshape weights ----
    S = x.sum(axis=1)                               # (B, D_IN)
    u = np.concatenate([S - x[:, T - 1, :], S, S - x[:, 0, :]], axis=1)  # (B, KDIM)
    u_in = np.ascontiguousarray(u.T.reshape(KT, 128, B))
    wencT = conv_w.transpose(2, 1, 0).reshape(KDIM, D_SAE)   # [j*D_IN+c, s]

    enc = _get("enc")
    in_maps = [
        {
            "u": u_in,
            "wenc": np.ascontiguousarray(wencT[:, m * S_SHARD : (m + 1) * S_SHARD]),
        }
        for m in core_ids
    ]
    res = run_bass_kernel_spmd(enc, in_maps, core_ids)
    pre = np.concatenate(
        [res.results[m]["preT"].transpose(2, 0, 1).reshape(B, S_SHARD) for m in core_ids],
        axis=1,
    )
    pre = pre + b_enc[None, :]                      # (B, D_SAE)

    # ---- host: exact top-K selection (order-invariant scatter) ----
    idx = np.argpartition(pre, D_SAE - K, axis=1)[:, -K:]    # (B, K) unordered
    vals = np.take_along_axis(pre, idx, axis=1)
    zvals = np.maximum(vals, 0.0).astype(np.float32)
    z = np.zeros_like(pre)
    np.put_along_axis(z, idx, zvals, axis=1)

    # ---- decode launch: batch-sharded sparse matmul over gathered rows ----
    dec = _get("dec")
    in_maps = []
    for m in core_ids:
        bb = slice(m * B_SHARD, (m + 1) * B_SHARD)
        A = W_dec[idx[bb].ravel()].reshape(ROWS, DFLAT)      # gathered rows
        rows = np.ascontiguousarray(
            A.reshape(ROWS, NGROUP, CHUNK_G * 128).transpose(1, 0, 2)
        )
        zd = np.zeros((ROWS, B_SHARD), dtype=np.float32)
        for b in range(B_SHARD):
            zd[b * K : (b + 1) * K, b] = zvals[m * B_SHARD + b]
        in_maps.append({"rows": rows, "zdiag": zd})
    res = run_bass_kernel_spmd(dec, in_maps, core_ids)
    x_hat = np.concatenate(
        [res.results[m]["xh"].transpose(2, 1, 0).reshape(B_SHARD, DFLAT) for m in core_ids],
        axis=0,
    ).reshape(B, T, D_IN)
    x_hat = x_hat + b_dec[None, :, :]

    recon_loss = np.float32(np.mean(np.sum((x_hat - x) ** 2, axis=-1)))
    return recon_loss, x_hat, z


# revision 8
# speedup vs baseline: 1.0958x; 1.0958x over previous
"""Trainium2 Bass kernel for the conv-encoder TopK SAE problem.

Math: the reference's conv1d('same', KS=3) followed by sum-over-T pooling
collapses exactly into one matmul:
    pre[b,s] = sum_{j,c} conv_w[s,c,j] * u_j[b,c] + b_enc[s]
with u_1 = sum_t x[b,t,c], u_0 = u_1 - x[b,T-1,c], u_2 = u_1 - x[b,0,c]
(the j-th tap sees the T-window shifted by j-1 under zero padding).

Distribution (8 cores):
  Launch 1 (encode): d_sae sharded; core m holds WencT[:, m*2048:(m+1)*2048]
    (WencT = conv_w reshaped to [3*768, 16384]) and computes
    pre_local = u @ WencT_m as 288 fp32 matmuls with the big operand on the
    LDWEIGHTS (stationary) path. DMA-bound: ~19 MB/core.
  Host: exact top-K of pre (2 MB), builds z, picks the <=B*K needed W_dec rows.
  Launch 2 (decode): batch sharded; core m decodes batches 4m..4m+3 as
    block-diagonal matmuls over only the gathered W_dec rows (6.3 MB/core
    instead of 100 MB/core for a dense decode).
"""

import numpy as np

import concourse.bacc as bacc
import concourse.mybir as mybir
from concourse.tile import TileContext
from concourse.bass_utils import run_bass_kernel_spmd

B, T, D_IN, D_SAE, K, KS = 32, 16, 768, 16384, 32, 3
NCORES = 8
S_SHARD = D_SAE // NCORES          # 2048 features per core
KDIM = KS * D_IN                   # 2304 contraction dim
KT = KDIM // 128                   # 18 k-tiles
ST = S_SHARD // 128                # 16 s-tiles per core
B_SHARD = B // NCORES              # 4 batches per core (decode)
ROWS = B_SHARD * K                 # 128 gathered rows per core (decode)
DFLAT = T * D_IN                   # 12288 decoded features per batch
NCHUNK = DFLAT // 128              # 96 column chunks (decode)
CHUNK_G = 8                        # chunks per DMA group (decode)
NGROUP = NCHUNK // CHUNK_G         # 12 DMA groups (decode)

F32 = mybir.dt.float32

_built = {}


def _build_encode():
    nc = bacc.Bacc("TRN2", target_bir_lowering=False, debug=False, num_devices=NCORES)
    u_d = nc.dram_tensor("u", [KT, 128, B], F32, kind="ExternalInput")
    w_d = nc.dram_tensor("wenc", [KDIM, S_SHARD], F32, kind="ExternalInput")
    o_d = nc.dram_tensor("preT", [128, ST, B], F32, kind="ExternalOutput")

    with TileContext(nc) as tc:
        with (
            tc.tile_pool(name="wp", bufs=1) as wp,
            tc.tile_pool(name="up", bufs=1) as up,
            tc.tile_pool(name="pp", bufs=1, space="PSUM") as pp,
        ):
            # front of the stream in 3-tile (3 MB) chunks for DMA efficiency,
            # the final k-tiles individually so the accumulation endgame is
            # not gated on one large trailing transfer.
            W_GROUPS = [3, 3, 3, 3, 3, 1, 1, 1]
            w_sb = []
            w_of = []
            k0 = 0
            for g, sz in enumerate(W_GROUPS):
                w = wp.tile([128, sz, S_SHARD], F32, tag=f"w{g}", name=f"w{g}")
                nc.sync.dma_start(
                    w[:],
                    w_d[k0 * 128 : (k0 + sz) * 128, :].rearrange(
                        "(t p) c -> p t c", p=128
                    ),
                )
                for t in range(sz):
                    w_sb.append(w)
                    w_of.append(t)
                k0 += sz

            u_sb = up.tile([128, KT, B], F32, tag="u", name="u_sb")
            nc.scalar.dma_start(u_sb[:], u_d[:].rearrange("t p b -> p t b"))

            # 16 accumulation groups packed 2-per-PSUM-bank so all 16 run
            # concurrently and PE consumes each weight tile as it arrives.
            # Within a bank only the FIRST matmul uses start=True (clears the
            # bank's has_written bits); the partner group's k=0 matmul then
            # overwrites (bits clear) and k>0 accumulates. Same-bank matmuls
            # keep program order under Tile's bank-level dep tracking.
            ps = [pp.tile([128, 2, B], F32, tag=f"ps{j}", name=f"ps{j}") for j in range(8)]
            for k in range(KT):
                for j in range(8):
                    for half in range(2):
                        s = j * 2 + half
                        nc.tensor.matmul(
                            ps[j][:, half, :],
                            w_sb[k][:, w_of[k], s * 128 : (s + 1) * 128],
                            u_sb[:, k, :],
                            start=(k == 0 and half == 0),
                            stop=(k == KT - 1 and half == 1),
                            skip_group_check=True,
                        )
            out_sb = up.tile([128, ST, B], F32, tag="o", name="out_sb")
            for j in range(8):
                nc.vector.tensor_copy(out_sb[:, j * 2 : (j + 1) * 2, :], ps[j][:])
            nc.scalar.dma_start(o_d[:], out_sb[:])

    nc.compile()
    return nc


def _build_decode():
    nc = bacc.Bacc("TRN2", target_bir_lowering=False, debug=False, num_devices=NCORES)
    r_d = nc.dram_tensor("rows", [NGROUP, 128, CHUNK_G * 128], F32, kind="ExternalInput")
    z_d = nc.dram_tensor("zdiag", [ROWS, B_SHARD], F32, kind="ExternalInput")
    o_d = nc.dram_tensor("xh", [128, NCHUNK, B_SHARD], F32, kind="ExternalOutput")

    with TileContext(nc) as tc:
        with (
            tc.tile_pool(name="rp", bufs=1) as rp,
            tc.tile_pool(name="sp", bufs=1) as sp,
            tc.tile_pool(name="pp", bufs=8, space="PSUM") as pp,
        ):
            z_sb = sp.tile([ROWS, B_SHARD], F32, tag="z", name="z_sb")
            nc.scalar.dma_start(z_sb[:], z_d[:])

            r_sb = []
            for g in range(NGROUP):
                r = rp.tile([128, CHUNK_G * 128], F32, tag=f"r{g}", name=f"r{g}")
                nc.sync.dma_start(r[:], r_d[g])
                r_sb.append(r)

            for g in range(NGROUP):
                out_g = sp.tile([128, CHUNK_G, B_SHARD], F32, tag=f"og{g}", name=f"out_g{g}")
                for i in range(CHUNK_G):
                    c = g * CHUNK_G + i
                    ps = pp.tile([128, B_SHARD], F32, tag="ps", name=f"ps{c}")
                    nc.tensor.matmul(
                        ps[:],
                        r_sb[g][:, i * 128 : (i + 1) * 128],
                        z_sb[:],
                        start=True,
                        stop=True,
                    )
                    nc.vector.tensor_copy(out_g[:, i, :], ps[:])
                nc.scalar.dma_start(o_d[:, g * CHUNK_G : (g + 1) * CHUNK_G, :], out_g[:])

    nc.compile()
    return nc


def _get(name):
    if name not in _built:
        _built[name] = _build_encode() if name == "enc" else _build_decode()
    return _built[name]


def kernel(x, conv_w, b_enc, W_dec, b_dec):
    x = np.ascontiguousarray(x, dtype=np.float32)
    conv_w = np.asarray(conv_w, dtype=np.float32)
    b_enc = np.asarray(b_enc, dtype=np.float32)
    W_dec = np.asarray(W_dec, dtype=np.float32)
    b_dec = np.asarray(b_dec, dtype=np.float32)
    core_ids = list(range(NCORES))

    # ---- host prep: collapse conv+pool into u, reshape weights ----
    S = x.sum(axis=1)                               # (B, D_IN)
    u = np.concatenate([S - x[:, T - 1, :], S, S - x[:, 0, :]], axis=1)  # (B, KDIM)
    u_in = np.ascontiguousarray(u.T.reshape(KT, 128, B))
    wencT = conv_w.transpose(2, 1, 0).reshape(KDIM, D_SAE)   # [j*D_IN+c, s]

    enc = _get("enc")
    in_maps = [
        {
            "u": u_in,
            "wenc": np.ascontiguousarray(wencT[:, m * S_SHARD : (m + 1) * S_SHARD]),
        }
        for m in core_ids
    ]
    res = run_bass_kernel_spmd(enc, in_maps, core_ids)
    pre = np.concatenate(
        [res.results[m]["preT"].transpose(2, 1, 0).reshape(B, S_SHARD) for m in core_ids],
        axis=1,
    )
    pre = pre + b_enc[None, :]                      # (B, D_SAE)

    # ---- host: exact top-K selection (order-invariant scatter) ----
    idx = np.argpartition(pre, D_SAE - K, axis=1)[:, -K:]    # (B, K) unordered
    vals = np.take_along_axis(pre, idx, axis=1)
    zvals = np.maximum(vals, 0.0).astype(np.float32)
    z = np.zeros_like(pre)
    np.put_along_axis(z, idx, zvals, axis=1)

    # ---- decode launch: batch-sharded sparse matmul over gathered rows ----
    dec = _get("dec")
    in_maps = []
    for m in core_ids:
        bb = slice(m * B_SHARD, (m + 1) * B_SHARD)
        A = W_dec[idx[bb].ravel()].reshape(ROWS, DFLAT)      # gathered rows
        rows = np.ascontiguousarray(
            A.reshape(ROWS, NGROUP, CHUNK_G * 128).transpose(1, 0, 2)
        )
        zd = np.zeros((ROWS, B_SHARD), dtype=np.float32)
        for b in range(B_SHARD):
            zd[b * K : (b + 1) * K, b] = zvals[m * B_SHARD + b]
        in_maps.append({"rows": rows, "zdiag": zd})
    res = run_bass_kernel_spmd(dec, in_maps, core_ids)
    x_hat = np.concatenate(
        [res.results[m]["xh"].transpose(2, 1, 0).reshape(B_SHARD, DFLAT) for m in core_ids],
        axis=0,
    ).reshape(B, T, D_IN)
    x_hat = x_hat + b_dec[None, :, :]

    recon_loss = np.float32(np.mean(np.sum((x_hat - x) ** 2, axis=-1)))
    return recon_loss, x_hat, z


# revision 9
# speedup vs baseline: 1.1049x; 1.0083x over previous
"""Trainium2 Bass kernel for the conv-encoder TopK SAE problem.

Math: the reference's conv1d('same', KS=3) followed by sum-over-T pooling
collapses exactly into one matmul:
    pre[b,s] = sum_{j,c} conv_w[s,c,j] * u_j[b,c] + b_enc[s]
with u_1 = sum_t x[b,t,c], u_0 = u_1 - x[b,T-1,c], u_2 = u_1 - x[b,0,c]
(the j-th tap sees the T-window shifted by j-1 under zero padding).

Distribution (8 cores):
  Launch 1 (encode): d_sae sharded; core m holds WencT[:, m*2048:(m+1)*2048]
    (WencT = conv_w reshaped to [3*768, 16384]) and computes
    pre_local = u @ WencT_m as 288 fp32 matmuls with the big operand on the
    LDWEIGHTS (stationary) path. DMA-bound: ~19 MB/core.
  Host: exact top-K of pre (2 MB), builds z, picks the <=B*K needed W_dec rows.
  Launch 2 (decode): batch sharded; core m decodes batches 4m..4m+3 as
    block-diagonal matmuls over only the gathered W_dec rows (6.3 MB/core
    instead of 100 MB/core for a dense decode).
"""

import numpy as np

import concourse.bacc as bacc
import concourse.mybir as mybir
from concourse.tile import TileContext
from concourse.bass_utils import run_bass_kernel_spmd

B, T, D_IN, D_SAE, K, KS = 32, 16, 768, 16384, 32, 3
NCORES = 8
S_SHARD = D_SAE // NCORES          # 2048 features per core
KDIM = KS * D_IN                   # 2304 contraction dim
KT = KDIM // 128                   # 18 k-tiles
ST = S_SHARD // 128                # 16 s-tiles per core
B_SHARD = B // NCORES              # 4 batches per core (decode)
ROWS = B_SHARD * K                 # 128 gathered rows per core (decode)
DFLAT = T * D_IN                   # 12288 decoded features per batch
NCHUNK = DFLAT // 128              # 96 column chunks (decode)
CHUNK_G = 8                        # chunks per DMA group (decode)
NGROUP = NCHUNK // CHUNK_G         # 12 DMA groups (decode)

F32 = mybir.dt.float32

_built = {}


def _build_encode():
    nc = bacc.Bacc("TRN2", target_bir_lowering=False, debug=False, num_devices=NCORES)
    u_d = nc.dram_tensor("u", [KT, 128, B], F32, kind="ExternalInput")
    w_d = nc.dram_tensor("wenc", [KDIM, S_SHARD], F32, kind="ExternalInput")
    o_d = nc.dram_tensor("preT", [128, ST, B], F32, kind="ExternalOutput")

    with TileContext(nc) as tc:
        with (
            tc.tile_pool(name="wp", bufs=1) as wp,
            tc.tile_pool(name="up", bufs=1) as up,
            tc.tile_pool(name="pp", bufs=1, space="PSUM") as pp,
        ):
            # front of the stream in 3-tile (3 MB) chunks for DMA efficiency,
            # the final k-tiles individually so the accumulation endgame is
            # not gated on one large trailing transfer.
            W_GROUPS = [3, 3, 3, 3, 3, 1, 1, 1]
            w_sb = []
            w_of = []
            k0 = 0
            for g, sz in enumerate(W_GROUPS):
                w = wp.tile([128, sz, S_SHARD], F32, tag=f"w{g}", name=f"w{g}")
                nc.sync.dma_start(
                    w[:],
                    w_d[k0 * 128 : (k0 + sz) * 128, :].rearrange(
                        "(t p) c -> p t c", p=128
                    ),
                )
                for t in range(sz):
                    w_sb.append(w)
                    w_of.append(t)
                k0 += sz

            u_sb = up.tile([128, KT, B], F32, tag="u", name="u_sb")
            nc.scalar.dma_start(u_sb[:], u_d[:].rearrange("t p b -> p t b"))

            # 16 accumulation groups packed 2-per-PSUM-bank so all 16 run
            # concurrently and PE consumes each weight tile as it arrives.
            # Within a bank only the FIRST matmul uses start=True (clears the
            # bank's has_written bits); the partner group's k=0 matmul then
            # overwrites (bits clear) and k>0 accumulates. Same-bank matmuls
            # keep program order under Tile's bank-level dep tracking.
            ps = [pp.tile([128, 2, B], F32, tag=f"ps{j}", name=f"ps{j}") for j in range(8)]
            for k in range(KT):
                for j in range(8):
                    for half in range(2):
                        s = j * 2 + half
                        nc.tensor.matmul(
                            ps[j][:, half, :],
                            w_sb[k][:, w_of[k], s * 128 : (s + 1) * 128],
                            u_sb[:, k, :],
                            start=(k == 0 and half == 0),
                            stop=(k == KT - 1 and half == 1),
                            skip_group_check=True,
                        )
            out_sb = up.tile([128, ST, B], F32, tag="o", name="out_sb")
            for j in range(8):
                nc.vector.tensor_copy(out_sb[:, j * 2 : (j + 1) * 2, :], ps[j][:])
            nc.scalar.dma_start(o_d[:], out_sb[:])

    nc.compile()
    return nc


def _build_decode():
    nc = bacc.Bacc("TRN2", target_bir_lowering=False, debug=False, num_devices=NCORES)
    r_d = nc.dram_tensor("rows", [NGROUP, 128, CHUNK_G * 128], F32, kind="ExternalInput")
    z_d = nc.dram_tensor("zdiag", [ROWS, B_SHARD], F32, kind="ExternalInput")
    o_d = nc.dram_tensor("xh", [128, NCHUNK, B_SHARD], F32, kind="ExternalOutput")

    with TileContext(nc) as tc:
        with (
            tc.tile_pool(name="rp", bufs=1) as rp,
            tc.tile_pool(name="sp", bufs=1) as sp,
            tc.tile_pool(name="pp", bufs=8, space="PSUM") as pp,
        ):
            z_sb = sp.tile([ROWS, B_SHARD], F32, tag="z", name="z_sb")
            nc.scalar.dma_start(z_sb[:], z_d[:])

            r_sb = []
            for g in range(NGROUP):
                r = rp.tile([128, CHUNK_G * 128], F32, tag=f"r{g}", name=f"r{g}")
                nc.sync.dma_start(r[:], r_d[g])
                r_sb.append(r)

            # 4 chunk outputs share one PSUM bank (start=True only on the
            # first; the rest overwrite into cleared has_written regions --
            # same bank-packing pattern as the encode) so each group drains
            # with 2 copies instead of 8.
            QUAD = 4
            for g in range(NGROUP):
                out_g = sp.tile([128, CHUNK_G, B_SHARD], F32, tag=f"og{g}", name=f"out_g{g}")
                for q in range(CHUNK_G // QUAD):
                    ps = pp.tile([128, QUAD, B_SHARD], F32, tag="ps", name=f"ps{g}_{q}")
                    for t in range(QUAD):
                        i = q * QUAD + t
                        nc.tensor.matmul(
                            ps[:, t, :],
                            r_sb[g][:, i * 128 : (i + 1) * 128],
                            z_sb[:],
                            start=(t == 0),
                            stop=(t == QUAD - 1),
                            skip_group_check=True,
                        )
                    nc.vector.tensor_copy(out_g[:, q * QUAD : (q + 1) * QUAD, :], ps[:])
                nc.scalar.dma_start(o_d[:, g * CHUNK_G : (g + 1) * CHUNK_G, :], out_g[:])

    nc.compile()
    return nc


def _get(name):
    if name not in _built:
        _built[name] = _build_encode() if name == "enc" else _build_decode()
    return _built[name]


def kernel(x, conv_w, b_enc, W_dec, b_dec):
    x = np.ascontiguousarray(x, dtype=np.float32)
    conv_w = np.asarray(conv_w, dtype=np.float32)
    b_enc = np.asarray(b_enc, dtype=np.float32)
    W_dec = np.asarray(W_dec, dtype=np.float32)
    b_dec = np.asarray(b_dec, dtype=np.float32)
    core_ids = list(range(NCORES))

    # ---- host prep: collapse conv+pool into u, reshape weights ----
    S = x.sum(axis=1)                               # (B, D_IN)
    u = np.concatenate([S - x[:, T - 1, :], S, S - x[:, 0, :]], axis=1)  # (B, KDIM)
    u_in = np.ascontiguousarray(u.T.reshape(KT, 128, B))
    wencT = conv_w.transpose(2, 1, 0).reshape(KDIM, D_SAE)   # [j*D_IN+c, s]

    enc = _get("enc")
    in_maps = [
        {
            "u": u_in,
            "wenc": np.ascontiguousarray(wencT[:, m * S_SHARD : (m + 1) * S_SHARD]),
        }
        for m in core_ids
    ]
    res = run_bass_kernel_spmd(enc, in_maps, core_ids)
    pre = np.concatenate(
        [res.results[m]["preT"].transpose(2, 1, 0).reshape(B, S_SHARD) for m in core_ids],
        axis=1,
    )
    pre = pre + b_enc[None, :]                      # (B, D_SAE)

    # ---- host: exact top-K selection (order-invariant scatter) ----
    idx = np.argpartition(pre, D_SAE - K, axis=1)[:, -K:]    # (B, K) unordered
    vals = np.take_along_axis(pre, idx, axis=1)
    zvals = np.maximum(vals, 0.0).astype(np.float32)
    z = np.zeros_like(pre)
    np.put_along_axis(z, idx, zvals, axis=1)

    # ---- decode launch: batch-sharded sparse matmul over gathered rows ----
    dec = _get("dec")
    in_maps = []
    for m in core_ids:
        bb = slice(m * B_SHARD, (m + 1) * B_SHARD)
        A = W_dec[idx[bb].ravel()].reshape(ROWS, DFLAT)      # gathered rows
        rows = np.ascontiguousarray(
            A.reshape(ROWS, NGROUP, CHUNK_G * 128).transpose(1, 0, 2)
        )
        zd = np.zeros((ROWS, B_SHARD), dtype=np.float32)
        for b in range(B_SHARD):
            zd[b * K : (b + 1) * K, b] = zvals[m * B_SHARD + b]
        in_maps.append({"rows": rows, "zdiag": zd})
    res = run_bass_kernel_spmd(dec, in_maps, core_ids)
    x_hat = np.concatenate(
        [res.results[m]["xh"].transpose(2, 1, 0).reshape(B_SHARD, DFLAT) for m in core_ids],
        axis=0,
    ).reshape(B, T, D_IN)
    x_hat = x_hat + b_dec[None, :, :]

    recon_loss = np.float32(np.mean(np.sum((x_hat - x) ** 2, axis=-1)))
    return recon_loss, x_hat, z
